# revision 10
# baseline (speedup 1.0000x reference)
"""Multi-head causal attention (B=2, S=2048, D=1024, H=16) on 8 NeuronCores.

Sharding: tensor-parallel over heads (2 heads/core, both batches on every
core). Each core computes q/k/v projections for its 2 heads, causal
attention, and a partial output projection (its 128 rows of W_proj); the
host sums the 8 partials and adds b_proj.

Device-side layout choices (all matmuls bf16 with fp32 PSUM accumulate):
 - x arrives pre-transposed from the host (xT [1024, 4096]) so every
   matmul has its contraction dim on partitions with zero on-chip
   transposes of x.
 - q, k are produced transposed ([2*64 head-dims, 4096 tokens]); scores
   are computed as ST = K @ Q^T ([keys, queries]) so softmax needs no
   row-max (scaled scores are O(6)) and exp(ST) feeds the AV matmul as
   the stationary operand directly.
 - v is produced token-major [tokens, 130] = [V_h0 | 1 | V_h1 | 1]; the
   ones columns come from a rank-1 (K=1) bias matmul, so the AV product
   expST.T @ [V|1] yields context AND the softmax denominator in one
   accumulation group, with queries on PSUM partitions -> normalization
   is a per-partition tensor_scalar multiply.
 - causal mask: multiply exp(scores) of the diagonal 128x128 block by a
   0/1 triangle (exact: masked entries contribute exactly 0, matching
   exp(-1e9/8) == 0 in fp32).
"""

import sys

sys.path.insert(0, "/opt/trn_rl_repo")

import numpy as np
import ml_dtypes

import concourse.bass as bass
import concourse.mybir as mybir
import concourse.tile as tile
from concourse import bacc
from concourse.bass_utils import run_bass_kernel_spmd

BF16 = mybir.dt.bfloat16
F32 = mybir.dt.float32
NPBF16 = ml_dtypes.bfloat16

B, S, D = 2, 2048, 1024
H, DH = 16, 64
T = B * S            # 4096 tokens
KS = D // 128        # 8 contraction subtiles
QT = S // 128        # 16 query tiles per batch
STRIP = 8            # kj blocks per psum strip (2 PSUM banks)
ACT_F = mybir.ActivationFunctionType

def _build_nc():
    # Bacc (not raw Bass): its compile() pass pipeline splits multi-sem
    # waits down to the TRN2 1-wait-per-instruction hardware limit.
    nc = bacc.Bacc("TRN2", target_bir_lowering=False, debug=False, num_devices=8)

    xT = nc.dram_tensor("xT", [D, T], BF16, kind="ExternalInput")
    wq = nc.dram_tensor("wq", [D, 128], BF16, kind="ExternalInput")
    wk = nc.dram_tensor("wk", [D, 128], BF16, kind="ExternalInput")
    wv = nc.dram_tensor("wv", [D, 130], BF16, kind="ExternalInput")
    bq = nc.dram_tensor("bq", [128, 1], F32, kind="ExternalInput")
    bk = nc.dram_tensor("bk", [128, 1], F32, kind="ExternalInput")
    bv = nc.dram_tensor("bv", [1, 130], BF16, kind="ExternalInput")
    wp = nc.dram_tensor("wp", [128, D], BF16, kind="ExternalInput")
    tri = nc.dram_tensor("tri", [128, 128], BF16, kind="ExternalInput")
    out = nc.dram_tensor("o", [T, D], BF16, kind="ExternalOutput")

    with tile.TileContext(nc) as tc:
        with (
            tc.tile_pool(name="singles", bufs=1) as singles,
            tc.tile_pool(name="pp", bufs=2, space="PSUM") as pp,
            tc.tile_pool(name="qkps", bufs=2, space="PSUM") as qkps,
            tc.tile_pool(name="avps", bufs=2, space="PSUM") as avps,
            tc.tile_pool(name="expp", bufs=12) as expp,
            tc.tile_pool(name="ctxp", bufs=8) as ctxp,
            tc.tile_pool(name="outp", bufs=3) as outp,
            tc.tile_pool(name="rdp", bufs=4) as rdp,
        ):
            # ---- resident tensors -------------------------------------
            wq_sb = singles.tile([128, KS, 128], BF16, tag="wq")
            wk_sb = singles.tile([128, KS, 128], BF16, tag="wk")
            wv_sb = singles.tile([128, KS, 130], BF16, tag="wv")
            bq_sb = singles.tile([128, 1], F32, tag="bq")
            bk_sb = singles.tile([128, 1], F32, tag="bk")
            bv_sb = singles.tile([1, 130], BF16, tag="bv")
            wp_sb = singles.tile([128, D], BF16, tag="wp")
            tri_sb = singles.tile([128, 128], BF16, tag="tri")
            ones_sb = singles.tile([1, 128], BF16, tag="ones")
            xT_sb = singles.tile([128, KS, T], BF16, tag="xT")
            qT_sb = singles.tile([128, T], BF16, tag="qT")
            kT_sb = singles.tile([128, T], BF16, tag="kT")
            # v, per (batch, key-tile): [V_h0 | 1 | V_h1 | 1]
            v_sb = singles.tile([128, B, QT, 130], BF16, tag="v")
            ctxT_sb = singles.tile([128, T // 128, 128], BF16, tag="ctxT")

            # PE warm-up: dep-free matmuls that run inside the initial
            # input-DMA shadow so the HAM clock gate opens (2.4 GHz)
            # before the first real matmul issues.
            warm_l = singles.tile([128, 128], BF16, tag="warm_l")
            warm_r = singles.tile([128, 512], BF16, tag="warm_r")
            nc.vector.memset(warm_l[:], 0.0)
            nc.vector.memset(warm_r[:], 0.0)
            warm_ps = qkps.tile([128, 2, 512], F32, tag="qk", name="warm_ps")
            for _ in range(48):
                nc.tensor.matmul(
                    warm_ps[:, 0, :], warm_l[:], warm_r[:], start=True, stop=True
                )

            nc.gpsimd.dma_start(wq_sb[:], wq.rearrange("(o p) m -> p o m", p=128))
            nc.gpsimd.dma_start(wk_sb[:], wk.rearrange("(o p) m -> p o m", p=128))
            xT_r = xT.rearrange("(o p) t -> p o t", p=128)
            nc.gpsimd.dma_start(
                xT_sb[:, :, bass.ds(0, 512)], xT_r[:, :, bass.ds(0, 512)]
            )
            nc.gpsimd.dma_start(wv_sb[:], wv.rearrange("(o p) m -> p o m", p=128))
            nc.gpsimd.dma_start(bq_sb[:], bq[:])
            nc.gpsimd.dma_start(bk_sb[:], bk[:])
            nc.gpsimd.dma_start(bv_sb[:], bv[:])
            nc.gpsimd.dma_start(wp_sb[:], wp[:])
            nc.gpsimd.dma_start(tri_sb[:], tri[:])
            nc.vector.memset(ones_sb[:], 1.0)

            # ---- phase P: projections ---------------------------------
            for tc8 in range(8):  # 512-token column tiles
                csl = bass.ds(tc8 * 512, 512)
                if tc8 > 0:
                    nc.gpsimd.dma_start(xT_sb[:, :, csl], xT_r[:, :, csl])

                ps_q = pp.tile([128, 512], F32, tag="proj")
                for ks in range(KS):
                    nc.tensor.matmul(
                        ps_q[:],
                        wq_sb[:, ks, :],
                        xT_sb[:, ks, csl],
                        start=(ks == 0),
                        stop=(ks == KS - 1),
                    )
                nc.scalar.activation(
                    qT_sb[:, csl], ps_q[:], ACT_F.Identity, bias=bq_sb[:], scale=1.0
                )

                ps_k = pp.tile([128, 512], F32, tag="proj")
                for ks in range(KS):
                    nc.tensor.matmul(
                        ps_k[:],
                        wk_sb[:, ks, :],
                        xT_sb[:, ks, csl],
                        start=(ks == 0),
                        stop=(ks == KS - 1),
                    )
                nc.scalar.activation(
                    kT_sb[:, csl], ps_k[:], ACT_F.Identity, bias=bk_sb[:], scale=1.0
                )

                for j in range(4):  # 128-token v tiles within the column tile
                    tt = tc8 * 4 + j
                    bb, kj = divmod(tt, QT)
                    ps_v_full = pp.tile([128, 512], F32, tag="proj", name="ps_v")
                    ps_v = ps_v_full[:, :130]
                    for ks in range(KS):
                        nc.tensor.matmul(
                            ps_v,
                            xT_sb[:, ks, bass.ds(tt * 128, 128)],
                            wv_sb[:, ks, :],
                            start=(ks == 0),
                            stop=False,
                        )
                    # rank-1 bias matmul: adds b_v and writes the ones columns
                    nc.tensor.matmul(
                        ps_v, ones_sb[:], bv_sb[:], start=False, stop=True
                    )
                    nc.vector.tensor_copy(v_sb[:, bb, kj, :], ps_v)

            # ---- phase A: causal attention ----------------------------
            # Query tiles processed in groups of 4 (512 queries) so each
            # QK matmul has a 512-wide moving operand (amortizes the
            # per-block LDWEIGHTS of the stationary kT tiles). Scores
            # land [keys, queries] in psum pairs of kj blocks; one Exp
            # per pair; AV consumes 128x128 bf16 slices.
            for bb in range(B):
                boff = bb * S
                for g in range(QT // 4):
                    nkj = 4 * g + 4  # kj blocks this group needs
                    gsl = bass.ds(boff + g * 512, 512)
                    ctxs = [
                        ctxp.tile([128, 128], BF16, tag="ctx", name=f"ctx_{r}")
                        for r in range(4)
                    ]
                    for h in range(2):
                        hsl = slice(64 * h, 64 * h + 64)
                        ex_tiles = []
                        for j in range(0, nkj, 2):  # kj pairs
                            qk = qkps.tile([128, 2, 512], F32, tag="qk")
                            for i2 in range(2):
                                kj = j + i2
                                nc.tensor.matmul(
                                    qk[:, i2, :],
                                    kT_sb[hsl, bass.ds(boff + kj * 128, 128)],
                                    qT_sb[hsl, gsl],
                                    start=True,
                                    stop=True,
                                )
                            ex = expp.tile([128, 2, 512], BF16, tag="exp")
                            # queries below kj are fully masked; skip them
                            rlo = max(0, j - 4 * g)
                            esl = bass.ds(rlo * 128, 512 - rlo * 128)
                            nc.scalar.activation(
                                ex[:, :, esl], qk[:, :, esl], ACT_F.Exp, scale=0.125
                            )
                            ex_tiles.append(ex)
                        for r in range(4):  # zero masked triangle on diagonal
                            qi = 4 * g + r
                            dsl = bass.ds(r * 128, 128)
                            exd = ex_tiles[qi // 2]
                            nc.vector.tensor_mul(
                                exd[:, qi % 2, dsl], exd[:, qi % 2, dsl], tri_sb[:]
                            )
                        for r in range(4):
                            qi = 4 * g + r
                            av = avps.tile([128, 65], F32, tag="av")
                            for kj in range(qi + 1):
                                nc.tensor.matmul(
                                    av[:],
                                    ex_tiles[kj // 2][
                                        :, kj % 2, bass.ds(r * 128, 128)
                                    ],
                                    v_sb[:, bb, kj, bass.ds(65 * h, 65)],
                                    start=(kj == 0),
                                    stop=(kj == qi),
                                )
                            rd = rdp.tile([128, 1], F32, tag="rd")
                            nc.vector.reciprocal(rd[:], av[:, 64:65])
                            nc.vector.tensor_scalar_mul(
                                ctxs[r][:, hsl], av[:, 0:64], rd[:]
                            )

                    for r in range(4):
                        nc.sync.dma_start(
                            ctxT_sb[:, bb * QT + 4 * g + r, :],
                            ctxs[r][:],
                            transpose=True,
                        )

            # ---- phase O: output projection (partial: this core's rows)
            for tt in range(T // 128):
                ot = outp.tile([128, D], BF16, tag="out")
                for half in range(2):
                    po = pp.tile([128, 512], F32, tag="proj")
                    nc.tensor.matmul(
                        po[:],
                        ctxT_sb[:, tt, :],
                        wp_sb[:, bass.ds(half * 512, 512)],
                        start=True,
                        stop=True,
                    )
                    osl = bass.ds(half * 512, 512)
                    if half == 0:
                        nc.vector.tensor_copy(ot[:, osl], po[:])
                    else:
                        nc.scalar.copy(ot[:, osl], po[:])
                nc.gpsimd.dma_start(out[bass.ds(tt * 128, 128), :], ot[:])

    return nc


_NC_CACHE = None


def _get_nc():
    global _NC_CACHE
    if _NC_CACHE is None:
        nc = _build_nc()
        nc.finalize()  # runs Bacc's pass pipeline (sync-wait splitting etc.)
        _NC_CACHE = nc
    return _NC_CACHE


def _make_in_maps(x, W_qkv, b_qkv, W_proj):
    xT = np.ascontiguousarray(
        x.reshape(T, D).T.astype(NPBF16)
    )
    tri = np.triu(np.ones((128, 128), dtype=np.float32)).astype(NPBF16)

    in_maps = []
    for c in range(8):
        cs = slice(128 * c, 128 * c + 128)
        wq = np.ascontiguousarray(W_qkv[:, 0 * D :][:, cs].astype(NPBF16))
        wk = np.ascontiguousarray(W_qkv[:, 1 * D :][:, cs].astype(NPBF16))
        v_blk = W_qkv[:, 2 * D :][:, cs].astype(np.float32)
        wv = np.zeros((D, 130), dtype=np.float32)
        wv[:, 0:64] = v_blk[:, 0:64]
        wv[:, 65:129] = v_blk[:, 64:128]
        bv = np.zeros((1, 130), dtype=np.float32)
        bv[0, 0:64] = b_qkv[2 * D :][cs][0:64]
        bv[0, 65:129] = b_qkv[2 * D :][cs][64:128]
        bv[0, 64] = 1.0
        bv[0, 129] = 1.0
        in_maps.append(
            {
                "xT": xT,
                "wq": wq,
                "wk": wk,
                "wv": wv.astype(NPBF16),
                "bq": np.ascontiguousarray(
                    b_qkv[0 * D :][cs].astype(np.float32).reshape(128, 1)
                ),
                "bk": np.ascontiguousarray(
                    b_qkv[1 * D :][cs].astype(np.float32).reshape(128, 1)
                ),
                "bv": bv.astype(NPBF16),
                "wp": np.ascontiguousarray(W_proj[cs, :].astype(NPBF16)),
                "tri": tri,
            }
        )
    return in_maps


def kernel(x, W_qkv, b_qkv, W_proj, b_proj, **run_kwargs):
    x = np.asarray(x, dtype=np.float32)
    W_qkv = np.asarray(W_qkv, dtype=np.float32)
    b_qkv = np.asarray(b_qkv, dtype=np.float32)
    W_proj = np.asarray(W_proj, dtype=np.float32)
    b_proj = np.asarray(b_proj, dtype=np.float32)

    nc = _get_nc()
    in_maps = _make_in_maps(x, W_qkv, b_qkv, W_proj)
    res = run_bass_kernel_spmd(nc, in_maps, core_ids=list(range(8)), **run_kwargs)

    acc = np.zeros((T, D), dtype=np.float32)
    for c in range(8):
        acc += res.results[c]["o"].astype(np.float32)
    acc += b_proj[None, :]
    out = acc.reshape(B, S, D)
    kernel.last_result = res
    return out


# revision 18
# speedup vs baseline: 1.0072x; 1.0072x over previous
"""Multi-head causal attention (B=2, S=2048, D=1024, H=16) on 8 NeuronCores.

Sharding: tensor-parallel over heads (2 heads/core, both batches on every
core). Each core computes q/k/v projections for its 2 heads, causal
attention, and a partial output projection (its 128 rows of W_proj); the
host sums the 8 partials and adds b_proj.

Device-side layout choices (all matmuls bf16 with fp32 PSUM accumulate):
 - x arrives pre-transposed from the host (xT [1024, 4096]) so every
   matmul has its contraction dim on partitions with zero on-chip
   transposes of x.
 - q, k are produced transposed ([2*64 head-dims, 4096 tokens]); scores
   are computed as ST = K @ Q^T ([keys, queries]) so softmax needs no
   row-max (scaled scores are O(6)) and exp(ST) feeds the AV matmul as
   the stationary operand directly.
 - v is produced token-major [tokens, 130] = [V_h0 | 1 | V_h1 | 1]; the
   ones columns come from a rank-1 (K=1) bias matmul, so the AV product
   expST.T @ [V|1] yields context AND the softmax denominator in one
   accumulation group, with queries on PSUM partitions -> normalization
   is a per-partition tensor_scalar multiply.
 - causal mask: multiply exp(scores) of the diagonal 128x128 block by a
   0/1 triangle (exact: masked entries contribute exactly 0, matching
   exp(-1e9/8) == 0 in fp32).
"""

import sys

sys.path.insert(0, "/opt/trn_rl_repo")

import numpy as np
import ml_dtypes

import concourse.bass as bass
import concourse.mybir as mybir
import concourse.tile as tile
from concourse import bacc
from concourse.bass_utils import run_bass_kernel_spmd

BF16 = mybir.dt.bfloat16
F32 = mybir.dt.float32
NPBF16 = ml_dtypes.bfloat16

B, S, D = 2, 2048, 1024
H, DH = 16, 64
T = B * S            # 4096 tokens
KS = D // 128        # 8 contraction subtiles
QT = S // 128        # 16 query tiles per batch
STRIP = 8            # kj blocks per psum strip (2 PSUM banks)
ACT_F = mybir.ActivationFunctionType

def _build_nc():
    # Bacc (not raw Bass): its compile() pass pipeline splits multi-sem
    # waits down to the TRN2 1-wait-per-instruction hardware limit.
    nc = bacc.Bacc("TRN2", target_bir_lowering=False, debug=False, num_devices=8)

    xT = nc.dram_tensor("xT", [D, T], BF16, kind="ExternalInput")
    wq = nc.dram_tensor("wq", [D, 128], BF16, kind="ExternalInput")
    wk = nc.dram_tensor("wk", [D, 128], BF16, kind="ExternalInput")
    wv = nc.dram_tensor("wv", [D, 130], BF16, kind="ExternalInput")
    bq = nc.dram_tensor("bq", [128, 1], F32, kind="ExternalInput")
    bk = nc.dram_tensor("bk", [128, 1], F32, kind="ExternalInput")
    bv = nc.dram_tensor("bv", [1, 130], BF16, kind="ExternalInput")
    wp = nc.dram_tensor("wp", [128, D], BF16, kind="ExternalInput")
    tri = nc.dram_tensor("tri", [128, 128], BF16, kind="ExternalInput")
    out = nc.dram_tensor("o", [T, D], BF16, kind="ExternalOutput")

    with tile.TileContext(nc) as tc:
        with (
            tc.tile_pool(name="singles", bufs=1) as singles,
            # one psum pool: tag "qk" [128,2,512] f32 = 2 banks x 3 bufs,
            # tag "av" [128,65] = 1 bank x 2 bufs -> exactly 8 banks
            tc.tile_pool(name="qkps", bufs=3, space="PSUM") as qkps,
            tc.tile_pool(name="expp", bufs=20) as expp,
            tc.tile_pool(name="ctxp", bufs=8) as ctxp,
            tc.tile_pool(name="outp", bufs=3) as outp,
            tc.tile_pool(name="rdp", bufs=4) as rdp,
        ):
            # ---- resident tensors -------------------------------------
            wq_sb = singles.tile([128, KS, 128], BF16, tag="wq")
            wk_sb = singles.tile([128, KS, 128], BF16, tag="wk")
            wv_sb = singles.tile([128, KS, 130], BF16, tag="wv")
            bq_sb = singles.tile([128, 1], F32, tag="bq")
            bk_sb = singles.tile([128, 1], F32, tag="bk")
            # b_v (+ the ones columns) broadcast to all partitions; fused
            # into the v copyback as a tensor_tensor add
            bv_sb = singles.tile([128, 130], BF16, tag="bv")
            wp_sb = singles.tile([128, D], BF16, tag="wp")
            tri_sb = singles.tile([128, 128], BF16, tag="tri")
            xT_sb = singles.tile([128, KS, T], BF16, tag="xT")
            qT_sb = singles.tile([128, T], BF16, tag="qT")
            kT_sb = singles.tile([128, T], BF16, tag="kT")
            # v, per (batch, key-tile): [V_h0 | 1 | V_h1 | 1]
            v_sb = singles.tile([128, B, QT, 130], BF16, tag="v")
            ctxT_sb = singles.tile([128, T // 128, 128], BF16, tag="ctxT")

            # PE warm-up: dep-free matmuls that run inside the initial
            # input-DMA shadow so the HAM clock gate opens (2.4 GHz)
            # before the first real matmul issues.
            warm_l = singles.tile([128, 128], BF16, tag="warm_l")
            warm_r = singles.tile([128, 512], BF16, tag="warm_r")
            nc.vector.memset(warm_l[:], 0.0)
            nc.vector.memset(warm_r[:], 0.0)
            warm_ps = qkps.tile([128, 2, 512], F32, tag="qk", name="warm_ps")
            for _ in range(48):
                nc.tensor.matmul(
                    warm_ps[:, 0, :], warm_l[:], warm_r[:], start=True, stop=True
                )

            nc.gpsimd.dma_start(wq_sb[:], wq.rearrange("(o p) m -> p o m", p=128))
            nc.gpsimd.dma_start(wk_sb[:], wk.rearrange("(o p) m -> p o m", p=128))
            xT_r = xT.rearrange("(o p) t -> p o t", p=128)
            nc.gpsimd.dma_start(
                xT_sb[:, :, bass.ds(0, 512)], xT_r[:, :, bass.ds(0, 512)]
            )
            nc.gpsimd.dma_start(wv_sb[:], wv.rearrange("(o p) m -> p o m", p=128))
            nc.gpsimd.dma_start(bq_sb[:], bq[:])
            nc.gpsimd.dma_start(bk_sb[:], bk[:])
            nc.gpsimd.dma_start(bv_sb[:], bv[:].to_broadcast((128, 130)))
            nc.gpsimd.dma_start(wp_sb[:], wp[:])
            nc.gpsimd.dma_start(tri_sb[:], tri[:])

            # ---- phase P: projections ---------------------------------
            for tc8 in range(8):  # 512-token column tiles
                csl = bass.ds(tc8 * 512, 512)
                if tc8 > 0:
                    nc.gpsimd.dma_start(xT_sb[:, :, csl], xT_r[:, :, csl])

                ps_qk = qkps.tile([128, 2, 512], F32, tag="qk", name="ps_qk")
                for ks in range(KS):
                    nc.tensor.matmul(
                        ps_qk[:, 0, :],
                        wq_sb[:, ks, :],
                        xT_sb[:, ks, csl],
                        start=(ks == 0),
                        stop=(ks == KS - 1),
                    )
                for ks in range(KS):
                    nc.tensor.matmul(
                        ps_qk[:, 1, :],
                        wk_sb[:, ks, :],
                        xT_sb[:, ks, csl],
                        start=(ks == 0),
                        stop=(ks == KS - 1),
                    )
                nc.scalar.activation(
                    qT_sb[:, csl],
                    ps_qk[:, 0, :],
                    ACT_F.Identity,
                    bias=bq_sb[:],
                    scale=1.0,
                )
                nc.scalar.activation(
                    kT_sb[:, csl],
                    ps_qk[:, 1, :],
                    ACT_F.Identity,
                    bias=bk_sb[:],
                    scale=1.0,
                )

                for jj in range(2):  # v tiles, two 128-token tiles per psum
                    ps_v = qkps.tile([128, 2, 512], F32, tag="qk", name="ps_v")
                    for i2 in range(2):
                        tt = tc8 * 4 + jj * 2 + i2
                        bb, kj = divmod(tt, QT)
                        for ks in range(KS):
                            nc.tensor.matmul(
                                ps_v[:, i2, :130],
                                xT_sb[:, ks, bass.ds(tt * 128, 128)],
                                wv_sb[:, ks, :],
                                start=(ks == 0),
                                stop=(ks == KS - 1),
                            )
                        # bias add also writes the ones columns (64, 129)
                        nc.vector.tensor_add(
                            v_sb[:, bb, kj, :], ps_v[:, i2, :130], bv_sb[:]
                        )

            # ---- phase A: causal attention ----------------------------
            # Query tiles processed in groups of 4 (512 queries) so each
            # QK matmul has a 512-wide moving operand (amortizes the
            # per-block LDWEIGHTS of the stationary kT tiles). Scores
            # land [keys, queries] in psum pairs of kj blocks; one Exp
            # per pair; AV consumes 128x128 bf16 slices.
            for bb in range(B):
                boff = bb * S
                for g in range(QT // 4):
                    nkj = 4 * g + 4  # kj blocks this group needs
                    gsl = bass.ds(boff + g * 512, 512)
                    ctxs = [
                        ctxp.tile([128, 128], BF16, tag="ctx", name=f"ctx_{r}")
                        for r in range(4)
                    ]
                    ex_tiles = [[], []]  # per head
                    for j in range(0, nkj, 2):  # kj pairs
                        qks = [
                            qkps.tile([128, 2, 512], F32, tag="qk", name=f"qk_h{h}")
                            for h in range(2)
                        ]
                        # interleave the two heads' K=64 matmuls so the PE
                        # can run them concurrently in disjoint row groups
                        for i2 in range(2):
                            kj = j + i2
                            ksl = bass.ds(boff + kj * 128, 128)
                            for h in range(2):
                                hsl = slice(64 * h, 64 * h + 64)
                                nc.tensor.matmul(
                                    qks[h][:, i2, :],
                                    kT_sb[hsl, ksl],
                                    qT_sb[hsl, gsl],
                                    start=True,
                                    stop=True,
                                )
                        # queries below kj are fully masked; skip them
                        rlo = max(0, j - 4 * g)
                        esl = bass.ds(rlo * 128, 512 - rlo * 128)
                        for h in range(2):
                            ex = expp.tile([128, 2, 512], BF16, tag="exp")
                            nc.scalar.activation(
                                ex[:, :, esl],
                                qks[h][:, :, esl],
                                ACT_F.Exp,
                                scale=0.125,
                            )
                            ex_tiles[h].append(ex)
                    for h in range(2):
                        for r in range(4):  # zero masked triangle on diagonal
                            qi = 4 * g + r
                            dsl = bass.ds(r * 128, 128)
                            exd = ex_tiles[h][qi // 2]
                            nc.vector.tensor_mul(
                                exd[:, qi % 2, dsl], exd[:, qi % 2, dsl], tri_sb[:]
                            )
                    for h in range(2):
                        hsl = slice(64 * h, 64 * h + 64)
                        for r in range(4):
                            qi = 4 * g + r
                            av = qkps.tile([128, 65], F32, tag="av", bufs=2)
                            for kj in range(qi + 1):
                                nc.tensor.matmul(
                                    av[:],
                                    ex_tiles[h][kj // 2][
                                        :, kj % 2, bass.ds(r * 128, 128)
                                    ],
                                    v_sb[:, bb, kj, bass.ds(65 * h, 65)],
                                    start=(kj == 0),
                                    stop=(kj == qi),
                                )
                            rd = rdp.tile([128, 1], F32, tag="rd")
                            nc.vector.reciprocal(rd[:], av[:, 64:65])
                            nc.vector.tensor_scalar_mul(
                                ctxs[r][:, hsl], av[:, 0:64], rd[:]
                            )

                    for r in range(4):
                        nc.sync.dma_start(
                            ctxT_sb[:, bb * QT + 4 * g + r, :],
                            ctxs[r][:],
                            transpose=True,
                        )

            # ---- phase O: output projection (partial: this core's rows)
            for tt in range(T // 128):
                ot = outp.tile([128, D], BF16, tag="out")
                po = qkps.tile([128, 2, 512], F32, tag="qk", name="ps_o")
                for half in range(2):
                    nc.tensor.matmul(
                        po[:, half, :],
                        ctxT_sb[:, tt, :],
                        wp_sb[:, bass.ds(half * 512, 512)],
                        start=True,
                        stop=True,
                    )
                    osl = bass.ds(half * 512, 512)
                    if half == 0:
                        nc.vector.tensor_copy(ot[:, osl], po[:, 0, :])
                    else:
                        nc.scalar.copy(ot[:, osl], po[:, 1, :])
                nc.gpsimd.dma_start(out[bass.ds(tt * 128, 128), :], ot[:])

    return nc


_NC_CACHE = None


def _get_nc():
    global _NC_CACHE
    if _NC_CACHE is None:
        nc = _build_nc()
        nc.finalize()  # runs Bacc's pass pipeline (sync-wait splitting etc.)
        _NC_CACHE = nc
    return _NC_CACHE


def _make_in_maps(x, W_qkv, b_qkv, W_proj):
    xT = np.ascontiguousarray(
        x.reshape(T, D).T.astype(NPBF16)
    )
    tri = np.triu(np.ones((128, 128), dtype=np.float32)).astype(NPBF16)

    in_maps = []
    for c in range(8):
        cs = slice(128 * c, 128 * c + 128)
        wq = np.ascontiguousarray(W_qkv[:, 0 * D :][:, cs].astype(NPBF16))
        wk = np.ascontiguousarray(W_qkv[:, 1 * D :][:, cs].astype(NPBF16))
        v_blk = W_qkv[:, 2 * D :][:, cs].astype(np.float32)
        wv = np.zeros((D, 130), dtype=np.float32)
        wv[:, 0:64] = v_blk[:, 0:64]
        wv[:, 65:129] = v_blk[:, 64:128]
        bv = np.zeros((1, 130), dtype=np.float32)
        bv[0, 0:64] = b_qkv[2 * D :][cs][0:64]
        bv[0, 65:129] = b_qkv[2 * D :][cs][64:128]
        bv[0, 64] = 1.0
        bv[0, 129] = 1.0
        in_maps.append(
            {
                "xT": xT,
                "wq": wq,
                "wk": wk,
                "wv": wv.astype(NPBF16),
                "bq": np.ascontiguousarray(
                    b_qkv[0 * D :][cs].astype(np.float32).reshape(128, 1)
                ),
                "bk": np.ascontiguousarray(
                    b_qkv[1 * D :][cs].astype(np.float32).reshape(128, 1)
                ),
                "bv": bv.astype(NPBF16),
                "wp": np.ascontiguousarray(W_proj[cs, :].astype(NPBF16)),
                "tri": tri,
            }
        )
    return in_maps


def kernel(x, W_qkv, b_qkv, W_proj, b_proj, **run_kwargs):
    x = np.asarray(x, dtype=np.float32)
    W_qkv = np.asarray(W_qkv, dtype=np.float32)
    b_qkv = np.asarray(b_qkv, dtype=np.float32)
    W_proj = np.asarray(W_proj, dtype=np.float32)
    b_proj = np.asarray(b_proj, dtype=np.float32)

    nc = _get_nc()
    in_maps = _make_in_maps(x, W_qkv, b_qkv, W_proj)
    res = run_bass_kernel_spmd(nc, in_maps, core_ids=list(range(8)), **run_kwargs)

    acc = np.zeros((T, D), dtype=np.float32)
    for c in range(8):
        acc += res.results[c]["o"].astype(np.float32)
    acc += b_proj[None, :]
    out = acc.reshape(B, S, D)
    kernel.last_result = res
    return out


# revision 22
# speedup vs baseline: 1.1519x; 1.1437x over previous
"""Multi-head causal attention (B=2, S=2048, D=1024, H=16) on 8 NeuronCores.

Sharding: tensor-parallel over heads (2 heads/core, both batches on every
core). Each core computes q/k/v projections for its 2 heads, causal
attention, and a partial output projection (its 128 rows of W_proj); the
host sums the 8 partials and adds b_proj.

Device-side layout choices (all matmuls bf16 with fp32 PSUM accumulate):
 - x arrives pre-transposed from the host (xT [1024, 4096]) so every
   matmul has its contraction dim on partitions with zero on-chip
   transposes of x.
 - q, k are produced transposed ([2*64 head-dims, 4096 tokens]); scores
   are computed as ST = K @ Q^T ([keys, queries]) so softmax needs no
   row-max (scaled scores are O(6)) and exp(ST) feeds the AV matmul as
   the stationary operand directly.
 - v is produced token-major [tokens, 130] = [V_h0 | 1 | V_h1 | 1]; the
   ones columns come from a rank-1 (K=1) bias matmul, so the AV product
   expST.T @ [V|1] yields context AND the softmax denominator in one
   accumulation group, with queries on PSUM partitions -> normalization
   is a per-partition tensor_scalar multiply.
 - causal mask: multiply exp(scores) of the diagonal 128x128 block by a
   0/1 triangle (exact: masked entries contribute exactly 0, matching
   exp(-1e9/8) == 0 in fp32).
"""

import sys

sys.path.insert(0, "/opt/trn_rl_repo")

import numpy as np
import ml_dtypes

import concourse.bass as bass
import concourse.mybir as mybir
import concourse.tile as tile
from concourse import bacc
from concourse.bass_utils import run_bass_kernel_spmd

BF16 = mybir.dt.bfloat16
F32 = mybir.dt.float32
NPBF16 = ml_dtypes.bfloat16

B, S, D = 2, 2048, 1024
H, DH = 16, 64
T = B * S            # 4096 tokens
KS = D // 128        # 8 contraction subtiles
QT = S // 128        # 16 query tiles per batch
STRIP = 8            # kj blocks per psum strip (2 PSUM banks)
ACT_F = mybir.ActivationFunctionType

def _build_nc():
    # Bacc (not raw Bass): its compile() pass pipeline splits multi-sem
    # waits down to the TRN2 1-wait-per-instruction hardware limit.
    nc = bacc.Bacc("TRN2", target_bir_lowering=False, debug=False, num_devices=8)

    xT = nc.dram_tensor("xT", [D, T], BF16, kind="ExternalInput")
    wq = nc.dram_tensor("wq", [D, 128], BF16, kind="ExternalInput")
    wk = nc.dram_tensor("wk", [D, 128], BF16, kind="ExternalInput")
    wv = nc.dram_tensor("wv", [D, 130], BF16, kind="ExternalInput")
    bq = nc.dram_tensor("bq", [128, 1], F32, kind="ExternalInput")
    bk = nc.dram_tensor("bk", [128, 1], F32, kind="ExternalInput")
    bv = nc.dram_tensor("bv", [1, 130], BF16, kind="ExternalInput")
    wp = nc.dram_tensor("wp", [128, D], BF16, kind="ExternalInput")
    tri = nc.dram_tensor("tri", [128, 128], BF16, kind="ExternalInput")
    out = nc.dram_tensor("o", [T, D], BF16, kind="ExternalOutput")

    with tile.TileContext(nc) as tc:
        with (
            tc.tile_pool(name="singles", bufs=1) as singles,
            # one psum pool: tag "qk" [128,2,512] f32 = 2 banks x 3 bufs,
            # tag "av" [128,65] = 1 bank x 2 bufs -> exactly 8 banks
            tc.tile_pool(name="qkps", bufs=3, space="PSUM") as qkps,
            tc.tile_pool(name="expp", bufs=20) as expp,
            tc.tile_pool(name="ctxp", bufs=8) as ctxp,
            tc.tile_pool(name="outp", bufs=3) as outp,
            tc.tile_pool(name="rdp", bufs=4) as rdp,
        ):
            # ---- resident tensors -------------------------------------
            wq_sb = singles.tile([128, KS, 128], BF16, tag="wq")
            wk_sb = singles.tile([128, KS, 128], BF16, tag="wk")
            wv_sb = singles.tile([128, KS, 130], BF16, tag="wv")
            bq_sb = singles.tile([128, 1], F32, tag="bq")
            bk_sb = singles.tile([128, 1], F32, tag="bk")
            # b_v (+ the ones columns) broadcast to all partitions; fused
            # into the v copyback as a tensor_tensor add
            bv_sb = singles.tile([128, 130], BF16, tag="bv")
            wp_sb = singles.tile([128, D], BF16, tag="wp")
            tri_sb = singles.tile([128, 128], BF16, tag="tri")
            xT_sb = singles.tile([128, KS, T], BF16, tag="xT")
            qT_sb = singles.tile([128, T], BF16, tag="qT")
            # kT stored twice, zero-padded per head: head h lives in rows
            # 64h..64h+63, other rows are 0. QK then contracts K=128 at
            # the full-array rate (K=64 matmuls measure ~2.5x slower);
            # the zero rows annihilate the other head's q rows.
            kTz = [
                singles.tile([128, T], BF16, tag=f"kTz{h}", name=f"kTz{h}")
                for h in range(2)
            ]
            # v, per (batch, key-tile): [V_h0 | 1 | V_h1 | 1]
            v_sb = singles.tile([128, B, QT, 130], BF16, tag="v")
            ctxT_sb = singles.tile([128, T // 128, 128], BF16, tag="ctxT")

            nc.vector.memset(kTz[0][64:128, :], 0.0)
            nc.vector.memset(kTz[1][0:64, :], 0.0)

            nc.gpsimd.dma_start(wq_sb[:], wq.rearrange("(o p) m -> p o m", p=128))
            nc.gpsimd.dma_start(wk_sb[:], wk.rearrange("(o p) m -> p o m", p=128))
            xT_r = xT.rearrange("(o p) t -> p o t", p=128)
            nc.gpsimd.dma_start(
                xT_sb[:, :, bass.ds(0, 512)], xT_r[:, :, bass.ds(0, 512)]
            )
            nc.gpsimd.dma_start(wv_sb[:], wv.rearrange("(o p) m -> p o m", p=128))
            nc.gpsimd.dma_start(bq_sb[:], bq[:])
            nc.gpsimd.dma_start(bk_sb[:], bk[:])
            nc.gpsimd.dma_start(bv_sb[:], bv[:].to_broadcast((128, 130)))
            nc.gpsimd.dma_start(wp_sb[:], wp[:])
            nc.gpsimd.dma_start(tri_sb[:], tri[:])

            # ---- phase P: projections ---------------------------------
            for tc8 in range(8):  # 512-token column tiles
                csl = bass.ds(tc8 * 512, 512)
                if tc8 > 0:
                    nc.gpsimd.dma_start(xT_sb[:, :, csl], xT_r[:, :, csl])

                ps_qk = qkps.tile([128, 2, 512], F32, tag="qk", name="ps_qk")
                for ks in range(KS):
                    nc.tensor.matmul(
                        ps_qk[:, 0, :],
                        wq_sb[:, ks, :],
                        xT_sb[:, ks, csl],
                        start=(ks == 0),
                        stop=(ks == KS - 1),
                    )
                for ks in range(KS):
                    nc.tensor.matmul(
                        ps_qk[:, 1, :],
                        wk_sb[:, ks, :],
                        xT_sb[:, ks, csl],
                        start=(ks == 0),
                        stop=(ks == KS - 1),
                    )
                nc.scalar.activation(
                    qT_sb[:, csl],
                    ps_qk[:, 0, :],
                    ACT_F.Identity,
                    bias=bq_sb[:],
                    scale=1.0,
                )
                nc.scalar.activation(
                    kTz[0][0:64, csl],
                    ps_qk[0:64, 1, :],
                    ACT_F.Identity,
                    bias=bk_sb[0:64],
                    scale=1.0,
                )
                nc.scalar.activation(
                    kTz[1][64:128, csl],
                    ps_qk[64:128, 1, :],
                    ACT_F.Identity,
                    bias=bk_sb[64:128],
                    scale=1.0,
                )

                for jj in range(2):  # v tiles, two 128-token tiles per psum
                    ps_v = qkps.tile([128, 2, 512], F32, tag="qk", name="ps_v")
                    for i2 in range(2):
                        tt = tc8 * 4 + jj * 2 + i2
                        bb, kj = divmod(tt, QT)
                        for ks in range(KS):
                            nc.tensor.matmul(
                                ps_v[:, i2, :130],
                                xT_sb[:, ks, bass.ds(tt * 128, 128)],
                                wv_sb[:, ks, :],
                                start=(ks == 0),
                                stop=(ks == KS - 1),
                            )
                        # bias add also writes the ones columns (64, 129)
                        nc.vector.tensor_add(
                            v_sb[:, bb, kj, :], ps_v[:, i2, :130], bv_sb[:]
                        )

            # ---- phase A: causal attention ----------------------------
            # Query tiles processed in groups of 4 (512 queries) so each
            # QK matmul has a 512-wide moving operand (amortizes the
            # per-block LDWEIGHTS of the stationary kT tiles). Scores
            # land [keys, queries] in psum pairs of kj blocks; one Exp
            # per pair; AV consumes 128x128 bf16 slices.
            for bb in range(B):
                boff = bb * S
                for g in range(QT // 4):
                    nkj = 4 * g + 4  # kj blocks this group needs
                    gsl = bass.ds(boff + g * 512, 512)
                    ctxs = [
                        ctxp.tile([128, 128], BF16, tag="ctx", name=f"ctx_{r}")
                        for r in range(4)
                    ]
                    ex_tiles = [[], []]  # per head
                    for j in range(0, nkj, 2):  # kj pairs
                        qks = [
                            qkps.tile([128, 2, 512], F32, tag="qk", name=f"qk_h{h}")
                            for h in range(2)
                        ]
                        for i2 in range(2):
                            kj = j + i2
                            ksl = bass.ds(boff + kj * 128, 128)
                            for h in range(2):
                                nc.tensor.matmul(
                                    qks[h][:, i2, :],
                                    kTz[h][:, ksl],
                                    qT_sb[:, gsl],
                                    start=True,
                                    stop=True,
                                )
                        # queries below kj are fully masked; skip them
                        rlo = max(0, j - 4 * g)
                        esl = bass.ds(rlo * 128, 512 - rlo * 128)
                        for h in range(2):
                            ex = expp.tile([128, 2, 512], BF16, tag="exp")
                            nc.scalar.activation(
                                ex[:, :, esl],
                                qks[h][:, :, esl],
                                ACT_F.Exp,
                                scale=0.125,
                            )
                            ex_tiles[h].append(ex)
                    for h in range(2):
                        for r in range(4):  # zero masked triangle on diagonal
                            qi = 4 * g + r
                            dsl = bass.ds(r * 128, 128)
                            exd = ex_tiles[h][qi // 2]
                            nc.vector.tensor_mul(
                                exd[:, qi % 2, dsl], exd[:, qi % 2, dsl], tri_sb[:]
                            )
                    for h in range(2):
                        hsl = slice(64 * h, 64 * h + 64)
                        for r in range(4):
                            qi = 4 * g + r
                            av = qkps.tile([128, 65], F32, tag="av", bufs=2)
                            for kj in range(qi + 1):
                                nc.tensor.matmul(
                                    av[:],
                                    ex_tiles[h][kj // 2][
                                        :, kj % 2, bass.ds(r * 128, 128)
                                    ],
                                    v_sb[:, bb, kj, bass.ds(65 * h, 65)],
                                    start=(kj == 0),
                                    stop=(kj == qi),
                                )
                            rd = rdp.tile([128, 1], F32, tag="rd")
                            nc.vector.reciprocal(rd[:], av[:, 64:65])
                            nc.vector.tensor_scalar_mul(
                                ctxs[r][:, hsl], av[:, 0:64], rd[:]
                            )

                    for r in range(4):
                        nc.sync.dma_start(
                            ctxT_sb[:, bb * QT + 4 * g + r, :],
                            ctxs[r][:],
                            transpose=True,
                        )

            # ---- phase O: output projection (partial: this core's rows)
            for tt in range(T // 128):
                ot = outp.tile([128, D], BF16, tag="out")
                po = qkps.tile([128, 2, 512], F32, tag="qk", name="ps_o")
                for half in range(2):
                    nc.tensor.matmul(
                        po[:, half, :],
                        ctxT_sb[:, tt, :],
                        wp_sb[:, bass.ds(half * 512, 512)],
                        start=True,
                        stop=True,
                    )
                    osl = bass.ds(half * 512, 512)
                    if half == 0:
                        nc.vector.tensor_copy(ot[:, osl], po[:, 0, :])
                    else:
                        nc.scalar.copy(ot[:, osl], po[:, 1, :])
                nc.gpsimd.dma_start(out[bass.ds(tt * 128, 128), :], ot[:])

    return nc


_NC_CACHE = None


def _get_nc():
    global _NC_CACHE
    if _NC_CACHE is None:
        nc = _build_nc()
        nc.finalize()  # runs Bacc's pass pipeline (sync-wait splitting etc.)
        _NC_CACHE = nc
    return _NC_CACHE


def _make_in_maps(x, W_qkv, b_qkv, W_proj):
    xT = np.ascontiguousarray(
        x.reshape(T, D).T.astype(NPBF16)
    )
    tri = np.triu(np.ones((128, 128), dtype=np.float32)).astype(NPBF16)

    in_maps = []
    for c in range(8):
        cs = slice(128 * c, 128 * c + 128)
        wq = np.ascontiguousarray(W_qkv[:, 0 * D :][:, cs].astype(NPBF16))
        wk = np.ascontiguousarray(W_qkv[:, 1 * D :][:, cs].astype(NPBF16))
        v_blk = W_qkv[:, 2 * D :][:, cs].astype(np.float32)
        wv = np.zeros((D, 130), dtype=np.float32)
        wv[:, 0:64] = v_blk[:, 0:64]
        wv[:, 65:129] = v_blk[:, 64:128]
        bv = np.zeros((1, 130), dtype=np.float32)
        bv[0, 0:64] = b_qkv[2 * D :][cs][0:64]
        bv[0, 65:129] = b_qkv[2 * D :][cs][64:128]
        bv[0, 64] = 1.0
        bv[0, 129] = 1.0
        in_maps.append(
            {
                "xT": xT,
                "wq": wq,
                "wk": wk,
                "wv": wv.astype(NPBF16),
                "bq": np.ascontiguousarray(
                    b_qkv[0 * D :][cs].astype(np.float32).reshape(128, 1)
                ),
                "bk": np.ascontiguousarray(
                    b_qkv[1 * D :][cs].astype(np.float32).reshape(128, 1)
                ),
                "bv": bv.astype(NPBF16),
                "wp": np.ascontiguousarray(W_proj[cs, :].astype(NPBF16)),
                "tri": tri,
            }
        )
    return in_maps


def kernel(x, W_qkv, b_qkv, W_proj, b_proj, **run_kwargs):
    x = np.asarray(x, dtype=np.float32)
    W_qkv = np.asarray(W_qkv, dtype=np.float32)
    b_qkv = np.asarray(b_qkv, dtype=np.float32)
    W_proj = np.asarray(W_proj, dtype=np.float32)
    b_proj = np.asarray(b_proj, dtype=np.float32)

    nc = _get_nc()
    in_maps = _make_in_maps(x, W_qkv, b_qkv, W_proj)
    res = run_bass_kernel_spmd(nc, in_maps, core_ids=list(range(8)), **run_kwargs)

    acc = np.zeros((T, D), dtype=np.float32)
    for c in range(8):
        acc += res.results[c]["o"].astype(np.float32)
    acc += b_proj[None, :]
    out = acc.reshape(B, S, D)
    kernel.last_result = res
    return out


# revision 24
# speedup vs baseline: 1.1769x; 1.0217x over previous
"""Multi-head causal attention (B=2, S=2048, D=1024, H=16) on 8 NeuronCores.

Sharding: tensor-parallel over heads (2 heads/core, both batches on every
core). Each core computes q/k/v projections for its 2 heads, causal
attention, and a partial output projection (its 128 rows of W_proj); the
host sums the 8 partials and adds b_proj.

Device-side layout choices (all matmuls bf16 with fp32 PSUM accumulate):
 - x arrives pre-transposed from the host (xT [1024, 4096]) so every
   matmul has its contraction dim on partitions with zero on-chip
   transposes of x.
 - q, k are produced transposed ([2*64 head-dims, 4096 tokens]); scores
   are computed as ST = K @ Q^T ([keys, queries]) so softmax needs no
   row-max (scaled scores are O(6)) and exp(ST) feeds the AV matmul as
   the stationary operand directly.
 - v is produced token-major [tokens, 130] = [V_h0 | 1 | V_h1 | 1]; the
   ones columns come from a rank-1 (K=1) bias matmul, so the AV product
   expST.T @ [V|1] yields context AND the softmax denominator in one
   accumulation group, with queries on PSUM partitions -> normalization
   is a per-partition tensor_scalar multiply.
 - causal mask: multiply exp(scores) of the diagonal 128x128 block by a
   0/1 triangle (exact: masked entries contribute exactly 0, matching
   exp(-1e9/8) == 0 in fp32).
"""

import sys

sys.path.insert(0, "/opt/trn_rl_repo")

import numpy as np
import ml_dtypes

import concourse.bass as bass
import concourse.mybir as mybir
import concourse.tile as tile
from concourse import bacc
from concourse.bass_utils import run_bass_kernel_spmd

BF16 = mybir.dt.bfloat16
F32 = mybir.dt.float32
NPBF16 = ml_dtypes.bfloat16

B, S, D = 2, 2048, 1024
H, DH = 16, 64
T = B * S            # 4096 tokens
KS = D // 128        # 8 contraction subtiles
QT = S // 128        # 16 query tiles per batch
STRIP = 8            # kj blocks per psum strip (2 PSUM banks)
ACT_F = mybir.ActivationFunctionType

def _build_nc():
    # Bacc (not raw Bass): its compile() pass pipeline splits multi-sem
    # waits down to the TRN2 1-wait-per-instruction hardware limit.
    nc = bacc.Bacc("TRN2", target_bir_lowering=False, debug=False, num_devices=8)

    xT = nc.dram_tensor("xT", [D, T], BF16, kind="ExternalInput")
    wq = nc.dram_tensor("wq", [D, 128], BF16, kind="ExternalInput")
    wk = nc.dram_tensor("wk", [D, 128], BF16, kind="ExternalInput")
    wv = nc.dram_tensor("wv", [D, 130], BF16, kind="ExternalInput")
    bq = nc.dram_tensor("bq", [128, 1], F32, kind="ExternalInput")
    bk = nc.dram_tensor("bk", [128, 1], F32, kind="ExternalInput")
    bv = nc.dram_tensor("bv", [1, 130], BF16, kind="ExternalInput")
    wp = nc.dram_tensor("wp", [128, D], BF16, kind="ExternalInput")
    tri = nc.dram_tensor("tri", [128, 128], BF16, kind="ExternalInput")
    out = nc.dram_tensor("o", [T, D], BF16, kind="ExternalOutput")

    with tile.TileContext(nc) as tc:
        with (
            tc.tile_pool(name="singles", bufs=1) as singles,
            # one psum pool: tag "qk" [128,2,512] f32 = 2 banks x 2 bufs,
            # tag "av" [128,65] = 1 bank x 2, tag "po" [128,512] = 1 bank
            # x 2 -> exactly 8 banks
            tc.tile_pool(name="qkps", bufs=2, space="PSUM") as qkps,
            tc.tile_pool(name="expp", bufs=20) as expp,
            tc.tile_pool(name="ctxp", bufs=8) as ctxp,
            tc.tile_pool(name="outp", bufs=3) as outp,
            tc.tile_pool(name="rdp", bufs=4) as rdp,
        ):
            # ---- resident tensors -------------------------------------
            wq_sb = singles.tile([128, KS, 128], BF16, tag="wq")
            wk_sb = singles.tile([128, KS, 128], BF16, tag="wk")
            wv_sb = singles.tile([128, KS, 130], BF16, tag="wv")
            bq_sb = singles.tile([128, 1], F32, tag="bq")
            bk_sb = singles.tile([128, 1], F32, tag="bk")
            # b_v (+ the ones columns) broadcast to all partitions; fused
            # into the v copyback as a tensor_tensor add
            bv_sb = singles.tile([128, 130], BF16, tag="bv")
            wp_sb = singles.tile([128, D], BF16, tag="wp")
            tri_sb = singles.tile([128, 128], BF16, tag="tri")
            xT_sb = singles.tile([128, KS, T], BF16, tag="xT")
            qT_sb = singles.tile([128, T], BF16, tag="qT")
            # kT stored twice, zero-padded per head: head h lives in rows
            # 64h..64h+63, other rows are 0. QK then contracts K=128 at
            # the full-array rate (K=64 matmuls measure ~2.5x slower);
            # the zero rows annihilate the other head's q rows.
            kTz = [
                singles.tile([128, T], BF16, tag=f"kTz{h}", name=f"kTz{h}")
                for h in range(2)
            ]
            # v, per (batch, key-tile): [V_h0 | 1 | V_h1 | 1]
            v_sb = singles.tile([128, B, QT, 130], BF16, tag="v")
            ctxT_sb = singles.tile([128, T // 128, 128], BF16, tag="ctxT")

            nc.vector.memset(kTz[0][64:128, :], 0.0)
            nc.vector.memset(kTz[1][0:64, :], 0.0)

            nc.gpsimd.dma_start(wq_sb[:], wq.rearrange("(o p) m -> p o m", p=128))
            nc.gpsimd.dma_start(wk_sb[:], wk.rearrange("(o p) m -> p o m", p=128))
            xT_r = xT.rearrange("(o p) t -> p o t", p=128)
            nc.gpsimd.dma_start(
                xT_sb[:, :, bass.ds(0, 512)], xT_r[:, :, bass.ds(0, 512)]
            )
            nc.gpsimd.dma_start(wv_sb[:], wv.rearrange("(o p) m -> p o m", p=128))
            nc.gpsimd.dma_start(bq_sb[:], bq[:])
            nc.gpsimd.dma_start(bk_sb[:], bk[:])
            nc.gpsimd.dma_start(bv_sb[:], bv[:].to_broadcast((128, 130)))
            nc.gpsimd.dma_start(wp_sb[:], wp[:])
            nc.gpsimd.dma_start(tri_sb[:], tri[:])

            # ---- phase P: projections ---------------------------------
            for tc8 in range(8):  # 512-token column tiles
                csl = bass.ds(tc8 * 512, 512)
                if tc8 > 0:
                    nc.gpsimd.dma_start(xT_sb[:, :, csl], xT_r[:, :, csl])

                ps_qk = qkps.tile([128, 2, 512], F32, tag="qk", name="ps_qk")
                for ks in range(KS):
                    nc.tensor.matmul(
                        ps_qk[:, 0, :],
                        wq_sb[:, ks, :],
                        xT_sb[:, ks, csl],
                        start=(ks == 0),
                        stop=(ks == KS - 1),
                    )
                for ks in range(KS):
                    nc.tensor.matmul(
                        ps_qk[:, 1, :],
                        wk_sb[:, ks, :],
                        xT_sb[:, ks, csl],
                        start=(ks == 0),
                        stop=(ks == KS - 1),
                    )
                nc.scalar.activation(
                    qT_sb[:, csl],
                    ps_qk[:, 0, :],
                    ACT_F.Identity,
                    bias=bq_sb[:],
                    scale=1.0,
                )
                nc.scalar.activation(
                    kTz[0][0:64, csl],
                    ps_qk[0:64, 1, :],
                    ACT_F.Identity,
                    bias=bk_sb[0:64],
                    scale=1.0,
                )
                nc.scalar.activation(
                    kTz[1][64:128, csl],
                    ps_qk[64:128, 1, :],
                    ACT_F.Identity,
                    bias=bk_sb[64:128],
                    scale=1.0,
                )

                for jj in range(2):  # v tiles, two 128-token tiles per psum
                    ps_v = qkps.tile([128, 2, 512], F32, tag="qk", name="ps_v")
                    for i2 in range(2):
                        tt = tc8 * 4 + jj * 2 + i2
                        bb, kj = divmod(tt, QT)
                        for ks in range(KS):
                            nc.tensor.matmul(
                                ps_v[:, i2, :130],
                                xT_sb[:, ks, bass.ds(tt * 128, 128)],
                                wv_sb[:, ks, :],
                                start=(ks == 0),
                                stop=(ks == KS - 1),
                            )
                        # bias add also writes the ones columns (64, 129)
                        nc.vector.tensor_add(
                            v_sb[:, bb, kj, :], ps_v[:, i2, :130], bv_sb[:]
                        )

            # ---- phase A: causal attention ----------------------------
            # Query tiles processed in groups of 4 (512 queries) so each
            # QK matmul has a 512-wide moving operand (amortizes the
            # per-block LDWEIGHTS of the stationary kT tiles). Scores
            # land [keys, queries] in psum pairs of kj blocks; one Exp
            # per pair; AV consumes 128x128 bf16 slices.
            for bb in range(B):
                boff = bb * S
                for g in range(QT // 4):
                    nkj = 4 * g + 4  # kj blocks this group needs
                    gsl = bass.ds(boff + g * 512, 512)
                    ctxs = [
                        ctxp.tile([128, 128], BF16, tag="ctx", name=f"ctx_{r}")
                        for r in range(4)
                    ]
                    ex_tiles = [[], []]  # per head
                    for j in range(0, nkj, 2):  # kj pairs
                        qks = [
                            qkps.tile([128, 2, 512], F32, tag="qk", name=f"qk_h{h}")
                            for h in range(2)
                        ]
                        for i2 in range(2):
                            kj = j + i2
                            ksl = bass.ds(boff + kj * 128, 128)
                            for h in range(2):
                                nc.tensor.matmul(
                                    qks[h][:, i2, :],
                                    kTz[h][:, ksl],
                                    qT_sb[:, gsl],
                                    start=True,
                                    stop=True,
                                )
                        # queries below kj are fully masked; skip them
                        rlo = max(0, j - 4 * g)
                        esl = bass.ds(rlo * 128, 512 - rlo * 128)
                        for h in range(2):
                            ex = expp.tile([128, 2, 512], BF16, tag="exp")
                            nc.scalar.activation(
                                ex[:, :, esl],
                                qks[h][:, :, esl],
                                ACT_F.Exp,
                                scale=0.125,
                            )
                            ex_tiles[h].append(ex)
                    for h in range(2):
                        for r in range(4):  # zero masked triangle on diagonal
                            qi = 4 * g + r
                            dsl = bass.ds(r * 128, 128)
                            exd = ex_tiles[h][qi // 2]
                            nc.vector.tensor_mul(
                                exd[:, qi % 2, dsl], exd[:, qi % 2, dsl], tri_sb[:]
                            )
                    for h in range(2):
                        hsl = slice(64 * h, 64 * h + 64)
                        for r in range(4):
                            qi = 4 * g + r
                            av = qkps.tile([128, 65], F32, tag="av", bufs=2)
                            for kj in range(qi + 1):
                                nc.tensor.matmul(
                                    av[:],
                                    ex_tiles[h][kj // 2][
                                        :, kj % 2, bass.ds(r * 128, 128)
                                    ],
                                    v_sb[:, bb, kj, bass.ds(65 * h, 65)],
                                    start=(kj == 0),
                                    stop=(kj == qi),
                                )
                            rd = rdp.tile([128, 1], F32, tag="rd")
                            nc.vector.reciprocal(rd[:], av[:, 64:65])
                            nc.vector.tensor_scalar_mul(
                                ctxs[r][:, hsl], av[:, 0:64], rd[:]
                            )

                    for r in range(4):
                        nc.sync.dma_start(
                            ctxT_sb[:, bb * QT + 4 * g + r, :],
                            ctxs[r][:],
                            transpose=True,
                        )

            # ---- phase O: output projection (partial: this core's rows)
            out_r = out.rearrange("(n p) d -> p n d", p=128)
            for t4 in range(T // 512):
                ot = outp.tile([128, 4, D], BF16, tag="out", bufs=2)
                for j4 in range(4):
                    tt = t4 * 4 + j4
                    for half in range(2):
                        po = qkps.tile(
                            [128, 512], F32, tag="po", name="ps_o", bufs=2
                        )
                        nc.tensor.matmul(
                            po[:],
                            ctxT_sb[:, tt, :],
                            wp_sb[:, bass.ds(half * 512, 512)],
                            start=True,
                            stop=True,
                        )
                        osl = bass.ds(half * 512, 512)
                        if half == 0:
                            nc.vector.tensor_copy(ot[:, j4, osl], po[:])
                        else:
                            nc.scalar.copy(ot[:, j4, osl], po[:])
                nc.gpsimd.dma_start(
                    out_r[:, bass.ds(t4 * 4, 4), :], ot[:]
                )

    return nc


_NC_CACHE = None


def _get_nc():
    global _NC_CACHE
    if _NC_CACHE is None:
        nc = _build_nc()
        nc.finalize()  # runs Bacc's pass pipeline (sync-wait splitting etc.)
        _NC_CACHE = nc
    return _NC_CACHE


def _make_in_maps(x, W_qkv, b_qkv, W_proj):
    xT = np.ascontiguousarray(
        x.reshape(T, D).T.astype(NPBF16)
    )
    tri = np.triu(np.ones((128, 128), dtype=np.float32)).astype(NPBF16)

    in_maps = []
    for c in range(8):
        cs = slice(128 * c, 128 * c + 128)
        wq = np.ascontiguousarray(W_qkv[:, 0 * D :][:, cs].astype(NPBF16))
        wk = np.ascontiguousarray(W_qkv[:, 1 * D :][:, cs].astype(NPBF16))
        v_blk = W_qkv[:, 2 * D :][:, cs].astype(np.float32)
        wv = np.zeros((D, 130), dtype=np.float32)
        wv[:, 0:64] = v_blk[:, 0:64]
        wv[:, 65:129] = v_blk[:, 64:128]
        bv = np.zeros((1, 130), dtype=np.float32)
        bv[0, 0:64] = b_qkv[2 * D :][cs][0:64]
        bv[0, 65:129] = b_qkv[2 * D :][cs][64:128]
        bv[0, 64] = 1.0
        bv[0, 129] = 1.0
        in_maps.append(
            {
                "xT": xT,
                "wq": wq,
                "wk": wk,
                "wv": wv.astype(NPBF16),
                "bq": np.ascontiguousarray(
                    b_qkv[0 * D :][cs].astype(np.float32).reshape(128, 1)
                ),
                "bk": np.ascontiguousarray(
                    b_qkv[1 * D :][cs].astype(np.float32).reshape(128, 1)
                ),
                "bv": bv.astype(NPBF16),
                "wp": np.ascontiguousarray(W_proj[cs, :].astype(NPBF16)),
                "tri": tri,
            }
        )
    return in_maps


def kernel(x, W_qkv, b_qkv, W_proj, b_proj, **run_kwargs):
    x = np.asarray(x, dtype=np.float32)
    W_qkv = np.asarray(W_qkv, dtype=np.float32)
    b_qkv = np.asarray(b_qkv, dtype=np.float32)
    W_proj = np.asarray(W_proj, dtype=np.float32)
    b_proj = np.asarray(b_proj, dtype=np.float32)

    nc = _get_nc()
    in_maps = _make_in_maps(x, W_qkv, b_qkv, W_proj)
    res = run_bass_kernel_spmd(nc, in_maps, core_ids=list(range(8)), **run_kwargs)

    acc = np.zeros((T, D), dtype=np.float32)
    for c in range(8):
        acc += res.results[c]["o"].astype(np.float32)
    acc += b_proj[None, :]
    out = acc.reshape(B, S, D)
    kernel.last_result = res
    return out
